# revision 8
# baseline (speedup 1.0000x reference)
"""Trainium2 Bass kernel for nn_MessagePassingCouplingLayer.

Strategy (data-parallel over batch, 2 batches per core x 8 cores):
  - Node features kept feature-major (hfT [H, NB]) in SBUF; message-layer
    linear parts a = hf @ Wm_top + bm, b = hf @ Wm_bot are computed per
    node tile with hfT slices as the stationary matmul operand (no
    transposes anywhere), written node-major to DRAM (bf16).
  - Per-edge gathers a[src], b[dst] via gpsimd dma_gather (8192 rows per
    instruction, 256B/row).  Messages m = relu(a_src + b_dst) are formed in
    PSUM with identity-matmul accumulation, relu'd on ACT into bf16.
  - segment_sum over dst is done per 128-node window: edges are host-sorted
    by destination window; a one-hot S[e, win] = (dst_local[e] == iota) is
    built on DVE per 128-edge tile and aggT[H, win] += m.T @ S accumulates
    in PSUM across the window's tiles.
  - Coupling MLPs run feature-major with folded weights (W_out folded into
    the first MLP layer, embedding folded into the input layer, temperature
    columns folded into biases).  Final tanh/exp/mask/log-det in f32.

The program is shared SPMD across the 8 cores, so per-(core,window) edge
counts are padded to the max over cores (padding edges carry dst_local=-1
which produces an all-zero one-hot column -> no contribution).
"""

import os
import numpy as np
import ml_dtypes

import concourse.bacc as bacc
import concourse.tile as tile
from concourse import mybir, bass_utils
from concourse.masks import make_identity
import concourse.bass as bass
from bass_rust import add_dep_helper

F32 = mybir.dt.float32
BF16 = mybir.dt.bfloat16
I16 = mybir.dt.int16
AF = mybir.ActivationFunctionType
OP = mybir.AluOpType

NCORES = 8
H = 128
WIN = 128          # aggregation window (nodes)
CH = 8192          # edges per dma_gather chunk
FT = 512           # node supertile width
NBF = ml_dtypes.bfloat16

LAST_EXEC_NS = None


def _bf(x):
    return np.ascontiguousarray(np.asarray(x, np.float32).astype(NBF))


def _f32(x):
    return np.ascontiguousarray(np.asarray(x, np.float32))


def _wrap_idx(idx, nchunk):
    """dma_gather index layout: per chunk, element i -> [i%16, i//16]."""
    per = CH // 16
    idx = idx.astype(np.int16).reshape(nchunk, per, 16)
    wrapped = np.transpose(idx, (0, 2, 1))          # [nchunk, 16, per]
    # the 8 GPSIMD Q7 cores each read their own 16-partition replica
    out = np.tile(wrapped, (1, 8, 1))               # [nchunk, 128, per]
    return np.ascontiguousarray(out)


def _host_prep(inputs):
    coords = _f32(inputs["coordinates"])          # [B,N,3]
    types = np.asarray(inputs["atom_types"])      # [B,N] int32
    adj = np.asarray(inputs["adj_list"])          # [E,2]
    eb = np.asarray(inputs["edge_batch_idx"])     # [E]
    masked = np.asarray(inputs["masked_elements"])
    assert not masked.any(), "kernel assumes no masked elements (spec fill=zeros)"

    B, N, _ = coords.shape
    V = int(np.asarray(inputs["embed"]).shape[0])
    BPC = B // NCORES
    NB = BPC * N
    NWIN = NB // WIN

    # ---- edge partitioning: per core, sorted/grouped by dst window ----
    core_of = eb // BPC
    half = eb % BPC
    src_g = half * N + adj[:, 0]
    dst_g = half * N + adj[:, 1]
    wog = dst_g // WIN

    counts = np.zeros((NCORES, NWIN), np.int64)
    per_core = []
    for c in range(NCORES):
        m = core_of == c
        s, d, w = src_g[m], dst_g[m], wog[m]
        order = np.argsort(w, kind="stable")
        per_core.append((s[order], d[order], w[order]))
        counts[c] = np.bincount(w, minlength=NWIN)

    tiles_w = np.maximum(1, -(-counts.max(axis=0) // 128))   # tiles per window
    len_w = tiles_w * 128
    EP0 = int(len_w.sum())
    EP = -(-EP0 // CH) * CH
    NT = EP // 128
    NCHUNK = EP // CH
    tail_tiles = (EP - EP0) // 128

    # tile -> window map + first/last flags (identical across cores)
    win_of, first, last = [], [], []
    for w in range(NWIN):
        for t in range(tiles_w[w]):
            win_of.append(w)
            first.append(t == 0)
            last.append(t == tiles_w[w] - 1 and not (w == NWIN - 1 and tail_tiles))
    for t in range(tail_tiles):
        win_of.append(NWIN - 1)
        first.append(False)
        last.append(t == tail_tiles - 1)

    starts = np.concatenate([[0], np.cumsum(len_w)[:-1]])

    meta = dict(B=B, N=N, V=V, BPC=BPC, NB=NB, NWIN=NWIN, EP=EP, NT=NT,
                NCHUNK=NCHUNK, win_of=win_of, first=first, last=last)

    # ---- folded weights (shared across cores) ----
    embed = _f32(inputs["embed"])            # [V,D]
    W_in = _f32(inputs["W_in"])              # [D+3,H]
    D = embed.shape[1]
    W_out = _f32(inputs["W_out"])
    b_out = _f32(inputs["b_out"])

    wmap = {
        "embW": _bf(embed @ W_in[:D]),                     # [V,H]
        "Winc": _bf(W_in[D:D + 3]),                        # [3,H]
        "b_in": _f32(inputs["b_in"]).reshape(H, 1),
    }
    for l in range(2):
        Wm = _f32(inputs[f"W_msg{l}"]); bm = _f32(inputs[f"b_msg{l}"])
        Wu = _f32(inputs[f"W_upd{l}"]); bu = _f32(inputs[f"b_upd{l}"])
        wmap[f"Wmcat{l}"] = _bf(np.concatenate([Wm[:H], Wm[H:]], axis=1))  # [H,2H]
        bmrow = np.zeros((1, 2 * H), np.float32)
        bmrow[0, :H] = bm
        wmap[f"bmrow{l}"] = _bf(bmrow)
        wmap[f"Wut{l}"] = _bf(Wu[:H])
        wmap[f"Wub{l}"] = _bf(Wu[H:])
        wmap[f"bu{l}"] = bu.reshape(H, 1)
    for p in ("s", "t"):
        W0 = _f32(inputs[f"W{p}0"]); b0 = _f32(inputs[f"b{p}0"])
        W1 = _f32(inputs[f"W{p}1"]); b1 = _f32(inputs[f"b{p}1"])
        W2 = _f32(inputs[f"W{p}2"]); b2 = _f32(inputs[f"b{p}2"])
        W3 = _f32(inputs[f"W{p}3"]); b3 = _f32(inputs[f"b{p}3"])
        wmap[f"W0a{p}"] = _bf(W_out @ W0[:H])
        wmap[f"W0c{p}"] = _bf(W0[H:H + 3])
        wmap[f"b0e{p}"] = (b0 + b_out @ W0[:H] + 300.0 * W0[H + 3] + 600.0 * W0[H + 4]).reshape(H, 1)
        wmap[f"W1{p}"] = _bf(W1); wmap[f"b1{p}"] = b1.reshape(H, 1)
        wmap[f"W2{p}"] = _bf(W2); wmap[f"b2{p}"] = b2.reshape(H // 2, 1)
        wmap[f"W3{p}"] = _bf(W3); wmap[f"b3{p}"] = b3.reshape(3, 1)

    # ---- per-core input maps ----
    in_maps = []
    onehot = np.eye(V, dtype=np.float32)
    for c in range(NCORES):
        b0 = c * BPC
        cflat = coords[b0:b0 + BPC].reshape(NB, 3)
        tflat = types[b0:b0 + BPC].reshape(NB)
        coupling = (tflat > 0).astype(np.float32)

        s, d, w = per_core[c]
        srcidx = np.zeros(EP, np.int64)
        dstloc = np.full(EP, -1.0, np.float32)
        dstidx = np.zeros(EP, np.int64)
        for wi in range(NWIN):
            lo = starts[wi]
            msk = w == wi
            k = int(msk.sum())
            srcidx[lo:lo + k] = s[msk]
            dstidx[lo:lo + k] = d[msk]
            dstloc[lo:lo + k] = (d[msk] - wi * WIN).astype(np.float32)

        im = dict(wmap)
        im["onehotT"] = _bf(onehot[tflat].T)                       # [V,NB]
        im["coordsT_bf"] = _bf(cflat.T)                            # [3,NB]
        im["condT_bf"] = _bf((cflat * (1.0 - coupling)[:, None]).T)
        im["coordsT_f"] = _f32(cflat.T)
        im["cmask3"] = _f32(np.broadcast_to(coupling, (3, NB)))
        im["idx_src"] = _wrap_idx(srcidx, NCHUNK)                  # [NCHUNK,128,CH/16]
        im["idx_dst"] = _wrap_idx(dstidx, NCHUNK)
        im["dstloc"] = np.ascontiguousarray(dstloc.reshape(NT, 128).T)
        in_maps.append(im)

    return meta, in_maps


def _build_program(meta):
    NB, NWIN, NT, NCHUNK, V = meta["NB"], meta["NWIN"], meta["NT"], meta["NCHUNK"], meta["V"]
    win_of, first, last = meta["win_of"], meta["first"], meta["last"]
    NSUP = NB // FT
    TPC = CH // 128          # tiles per chunk
    QPC = TPC // 4           # quads per chunk

    nc = bacc.Bacc("TRN2")

    # ---- DRAM I/O ----
    d_in = {}
    def din(name, shape, dt):
        d_in[name] = nc.dram_tensor(name, list(shape), dt, kind="ExternalInput")
        return d_in[name]

    din("onehotT", (V, NB), BF16)
    din("coordsT_bf", (3, NB), BF16)
    din("condT_bf", (3, NB), BF16)
    din("coordsT_f", (3, NB), F32)
    din("cmask3", (3, NB), F32)
    din("idx_src", (NCHUNK, 128, CH // 16), I16)
    din("idx_dst", (NCHUNK, 128, CH // 16), I16)
    din("dstloc", (128, NT), F32)
    din("embW", (V, H), BF16); din("Winc", (3, H), BF16); din("b_in", (H, 1), F32)
    for l in range(2):
        din(f"Wmcat{l}", (H, 2 * H), BF16); din(f"bmrow{l}", (1, 2 * H), BF16)
        din(f"Wut{l}", (H, H), BF16); din(f"Wub{l}", (H, H), BF16); din(f"bu{l}", (H, 1), F32)
    for p in ("s", "t"):
        din(f"W0a{p}", (H, H), BF16); din(f"W0c{p}", (3, H), BF16); din(f"b0e{p}", (H, 1), F32)
        din(f"W1{p}", (H, H), BF16); din(f"b1{p}", (H, 1), F32)
        din(f"W2{p}", (H, H // 2), BF16); din(f"b2{p}", (H // 2, 1), F32)
        din(f"W3{p}", (H // 2, 3), BF16); din(f"b3{p}", (3, 1), F32)

    outT_d = nc.dram_tensor("outT", [3, NB], F32, kind="ExternalOutput")
    ld_d = nc.dram_tensor("logdet", [1, meta["BPC"]], F32, kind="ExternalOutput")

    with tile.TileContext(nc) as tc:
        with (
            tc.tile_pool(name="const", bufs=1) as cpool,
            tc.tile_pool(name="hf", bufs=2) as hfpool,
            tc.tile_pool(name="agg", bufs=2) as aggpool,
            tc.tile_pool(name="gath", bufs=2) as gpool,
            tc.tile_pool(name="abgrp", bufs=2) as abpool,
            tc.tile_pool(name="work", bufs=3) as wpool,
            tc.tile_pool(name="sbig", bufs=2) as spool,
            tc.tile_pool(name="spool_S", bufs=4) as spool_S,
            tc.tile_pool(name="stream", bufs=2) as stpool,
            tc.tile_pool(name="dram", bufs=2, space="DRAM") as dpool,
            tc.tile_pool(name="ps_q", bufs=3, space="PSUM") as ps_q,
            tc.tile_pool(name="ps_agg", bufs=2, space="PSUM") as ps_agg,
            tc.tile_pool(name="ps_misc", bufs=3, space="PSUM") as ps_misc,
        ):
            # ---- constants in SBUF ----
            def load_const(name, shape, dt):
                t = cpool.tile(list(shape), dt, name=f"c_{name}")
                nc.sync.dma_start(t[:], d_in[name][:])
                return t

            W = {}
            for name in d_in:
                if name in ("onehotT", "coordsT_bf", "condT_bf", "coordsT_f",
                            "cmask3", "idx_src", "idx_dst"):
                    continue
                shp = d_in[name].shape
                W[name] = load_const(name, shp, d_in[name].dtype)

            iota_bf = cpool.tile([128, 128], BF16, name="iota_bf")
            nc.gpsimd.iota(iota_bf[:], pattern=[[1, 128]], base=0,
                           channel_multiplier=0, allow_small_or_imprecise_dtypes=True)
            ident = cpool.tile([128, 128], BF16, name="ident")
            make_identity(nc, ident[:])
            ones1 = cpool.tile([1, 128], BF16, name="ones1")
            nc.vector.memset(ones1[:], 1.0)
            ones31 = cpool.tile([3, 1], F32, name="ones31")
            nc.vector.memset(ones31[:], 1.0)
            ld_cols = cpool.tile([3, NSUP], F32, name="ld_cols")
            ld2 = cpool.tile([3, meta["BPC"]], F32, name="ld2")

            hfT = [hfpool.tile([128, NB], BF16, tag="hfT", name=f"hfT{i}") for i in range(3)]

            # ---- input layer ----
            for j in range(NSUP):
                sl = slice(j * FT, (j + 1) * FT)
                oh = stpool.tile([V, FT], BF16, tag="st_oh")
                nc.sync.dma_start(oh[:], d_in["onehotT"][:, sl])
                cb = stpool.tile([3, FT], BF16, tag="st_cb")
                nc.sync.dma_start(cb[:], d_in["coordsT_bf"][:, sl])
                ps = ps_q.tile([128, FT], F32, tag="psq")
                nc.tensor.matmul(ps[:], lhsT=W["embW"][:], rhs=oh[:], start=True, stop=False)
                nc.tensor.matmul(ps[:], lhsT=W["Winc"][:], rhs=cb[:], start=False, stop=True)
                nc.scalar.activation(hfT[0][:, sl], ps[:], AF.Relu, bias=W["b_in"][:])

            # ---- message-passing layers ----
            for l in range(2):
                hf_in, hf_out = hfT[l], hfT[l + 1]
                a_nm = dpool.tile([NB, H], BF16, tag="a_nm", name=f"a_nm{l}")
                b_nm = dpool.tile([NB, H], BF16, tag="b_nm", name=f"b_nm{l}")
                aggT = aggpool.tile([128, NB], BF16, tag="aggT", name=f"aggT{l}")

                # a/b tables, node-major, written in groups of GRP_T node tiles
                ab_writes = ([], [])
                GRP_T = min(16, NB // 128)
                NG = NB // (GRP_T * 128)
                for g in range(NG):
                    grp = abpool.tile([128, GRP_T, 256], BF16, tag="abgrp")
                    for jj in range(GRP_T):
                        j = g * GRP_T + jj
                        ps = ps_misc.tile([128, 256], F32, tag="ps_ab")
                        nc.tensor.matmul(ps[:], lhsT=hf_in[:, j * 128:(j + 1) * 128],
                                         rhs=W[f"Wmcat{l}"][:], start=True, stop=False)
                        nc.tensor.matmul(ps[:], lhsT=ones1[:], rhs=W[f"bmrow{l}"][:],
                                         start=False, stop=True)
                        nc.scalar.activation(grp[:, jj, :], ps[:], AF.Copy)
                    a_view = a_nm[:].rearrange("(g t p) c -> g p t c", g=NG, t=GRP_T, p=128)
                    b_view = b_nm[:].rearrange("(g t p) c -> g p t c", g=NG, t=GRP_T, p=128)
                    ab_writes[0].append(nc.sync.dma_start(a_view[g], grp[:, :, 0:128]).ins)
                    ab_writes[1].append(nc.sync.dma_start(b_view[g], grp[:, :, 128:256]).ins)

                # edge phase
                aggps = {}
                gathers = []
                for c in range(NCHUNK):
                    isrc = stpool.tile([128, CH // 16], I16, tag="isrc")
                    ld_a = nc.sync.dma_start(isrc[:], d_in["idx_src"][c])
                    idst = stpool.tile([128, CH // 16], I16, tag="idst")
                    ld_b = nc.sync.dma_start(idst[:], d_in["idx_dst"][c])
                    asrc = gpool.tile([128, TPC, 128], BF16, tag="asrc")
                    ga = nc.gpsimd.dma_gather(asrc[:], a_nm[:], isrc[:], num_idxs=CH,
                                              num_idxs_reg=CH, elem_size=H,
                                              single_packet=False)
                    bdst = gpool.tile([128, TPC, 128], BF16, tag="bdst")
                    gb = nc.gpsimd.dma_gather(bdst[:], b_nm[:], idst[:], num_idxs=CH,
                                              num_idxs_reg=CH, elem_size=H,
                                              single_packet=False)
                    add_dep_helper(ga.ins, ld_a.ins, reason="gather reads idx")
                    add_dep_helper(gb.ins, ld_b.ins, reason="gather reads idx")
                    for wi in ab_writes[0]:
                        add_dep_helper(ga.ins, wi, reason="gather after a_nm writes")
                    for wi in ab_writes[1]:
                        add_dep_helper(gb.ins, wi, reason="gather after b_nm writes")
                    gathers.append((ga, gb))
                    for tq in range(QPC):
                        mp = ps_q.tile([128, 512], F32, tag="psq")
                        mm_a = nc.tensor.matmul(mp[:], lhsT=ident[:],
                                                rhs=asrc[:, tq * 4:(tq + 1) * 4, :],
                                                start=True, stop=False)
                        mm_b = nc.tensor.matmul(mp[:], lhsT=ident[:],
                                                rhs=bdst[:, tq * 4:(tq + 1) * 4, :],
                                                start=False, stop=True)
                        add_dep_helper(mm_a.ins, ga.ins, reason="mm reads gathered a")
                        add_dep_helper(mm_b.ins, gb.ins, reason="mm reads gathered b")
                        m4 = wpool.tile([128, 512], BF16, tag="m4")
                        nc.scalar.activation(m4[:], mp[:], AF.Relu)
                        for q in range(4):
                            gt = c * TPC + tq * 4 + q
                            w = win_of[gt]
                            S = spool_S.tile([128, 128], BF16, tag="S")
                            nc.vector.tensor_scalar(
                                out=S[:], in0=iota_bf[:],
                                scalar1=W["dstloc"][:, gt:gt + 1], scalar2=None,
                                op0=OP.is_equal)
                            if first[gt]:
                                aggps[w] = ps_agg.tile([128, 128], F32, tag="agg", name=f"aggps_{l}_{w}")
                            nc.tensor.matmul(aggps[w][:], lhsT=m4[:, q * 128:(q + 1) * 128],
                                             rhs=S[:], start=first[gt], stop=last[gt])
                            if last[gt]:
                                nc.scalar.activation(aggT[:, w * 128:(w + 1) * 128],
                                                     aggps.pop(w)[:], AF.Copy)

                # update phase
                for j in range(NSUP):
                    sl = slice(j * FT, (j + 1) * FT)
                    ps = ps_q.tile([128, FT], F32, tag="psq")
                    nc.tensor.matmul(ps[:], lhsT=W[f"Wut{l}"][:], rhs=hf_in[:, sl],
                                     start=True, stop=False)
                    nc.tensor.matmul(ps[:], lhsT=W[f"Wub{l}"][:], rhs=aggT[:, sl],
                                     start=False, stop=True)
                    nc.scalar.activation(hf_out[:, sl], ps[:], AF.Relu, bias=W[f"bu{l}"][:])

            # ---- coupling MLPs + output ----
            hf2 = hfT[2]
            for j in range(NSUP):
                sl = slice(j * FT, (j + 1) * FT)
                cond = stpool.tile([3, FT], BF16, tag="st_cond")
                nc.sync.dma_start(cond[:], d_in["condT_bf"][:, sl])
                cmask = stpool.tile([3, FT], F32, tag="st_cm")
                nc.sync.dma_start(cmask[:], d_in["cmask3"][:, sl])
                cof = stpool.tile([3, FT], F32, tag="st_cof")
                nc.sync.dma_start(cof[:], d_in["coordsT_f"][:, sl])

                results = {}
                for p in ("s", "t"):
                    ps0 = ps_q.tile([128, FT], F32, tag="psq")
                    nc.tensor.matmul(ps0[:], lhsT=W[f"W0a{p}"][:], rhs=hf2[:, sl],
                                     start=True, stop=False)
                    nc.tensor.matmul(ps0[:], lhsT=W[f"W0c{p}"][:], rhs=cond[:],
                                     start=False, stop=True)
                    x1 = wpool.tile([128, FT], BF16, tag="mlp_x")
                    nc.scalar.activation(x1[:], ps0[:], AF.Relu, bias=W[f"b0e{p}"][:])
                    ps1 = ps_q.tile([128, FT], F32, tag="psq")
                    nc.tensor.matmul(ps1[:], lhsT=W[f"W1{p}"][:], rhs=x1[:],
                                     start=True, stop=True)
                    x2 = wpool.tile([128, FT], BF16, tag="mlp_x")
                    nc.scalar.activation(x2[:], ps1[:], AF.Relu, bias=W[f"b1{p}"][:])
                    ps2 = ps_misc.tile([H // 2, FT], F32, tag="ps_ab")
                    nc.tensor.matmul(ps2[:], lhsT=W[f"W2{p}"][:], rhs=x2[:],
                                     start=True, stop=True)
                    x3 = wpool.tile([H // 2, FT], BF16, tag="mlp_x3")
                    nc.scalar.activation(x3[:], ps2[:], AF.Relu, bias=W[f"b2{p}"][:])
                    ps3 = ps_misc.tile([3, FT], F32, tag="ps_ab")
                    nc.tensor.matmul(ps3[:], lhsT=W[f"W3{p}"][:], rhs=x3[:],
                                     start=True, stop=True)
                    if p == "s":
                        ls = spool.tile([3, FT], F32, tag="ls", name="ls")
                        nc.scalar.activation(ls[:], ps3[:], AF.Tanh, bias=W["b3s"][:])
                        nc.vector.tensor_tensor(out=ls[:], in0=ls[:], in1=cmask[:], op=OP.mult)
                        nc.vector.tensor_scalar_mul(ls[:], ls[:], 0.5)
                        nc.vector.tensor_reduce(ld_cols[:, j:j + 1], ls[:],
                                                axis=mybir.AxisListType.X, op=OP.add)
                    else:
                        sh = spool.tile([3, FT], F32, tag="sh", name="sh")
                        nc.vector.tensor_scalar(out=sh[:], in0=ps3[:], scalar1=W["b3t"][:],
                                                scalar2=None, op0=OP.add)
                        nc.vector.tensor_tensor(out=sh[:], in0=sh[:], in1=cmask[:], op=OP.mult)
                ex = spool.tile([3, FT], F32, tag="ex")
                nc.scalar.activation(ex[:], ls[:], AF.Exp)
                nc.vector.tensor_tensor(out=ex[:], in0=ex[:], in1=cof[:], op=OP.mult)
                nc.vector.tensor_tensor(out=ex[:], in0=ex[:], in1=sh[:], op=OP.add)
                nc.sync.dma_start(outT_d[:, sl], ex[:])

            # ---- log_det ----
            spb = NSUP // meta["BPC"]    # supertiles per batch
            for bb in range(meta["BPC"]):
                nc.vector.tensor_reduce(ld2[:, bb:bb + 1],
                                        ld_cols[:, bb * spb:(bb + 1) * spb],
                                        axis=mybir.AxisListType.X, op=OP.add)
            lps = ps_misc.tile([1, meta["BPC"]], F32, tag="ps_ab")
            nc.tensor.matmul(lps[:], lhsT=ones31[:], rhs=ld2[:], start=True, stop=True)
            ldo = spool.tile([1, meta["BPC"]], F32, tag="ldo")
            nc.scalar.activation(ldo[:], lps[:], AF.Copy)
            nc.sync.dma_start(ld_d[:], ldo[:])

    nc.compile()
    return nc


_CACHE = {}


def kernel(**inputs):
    global LAST_EXEC_NS
    meta, in_maps = _host_prep(inputs)
    key = (meta["B"], meta["N"], meta["EP"], meta["NT"], tuple(meta["win_of"]))
    if key not in _CACHE:
        _CACHE[key] = _build_program(meta)
    nc = _CACHE[key]

    trace = bool(os.environ.get("KERNEL_TRACE"))
    res = bass_utils.run_bass_kernel_spmd(nc, in_maps, core_ids=list(range(NCORES)),
                                          trace=trace)
    LAST_EXEC_NS = res.exec_time_ns

    B, N, BPC = meta["B"], meta["N"], meta["BPC"]
    out = np.empty((B, N, 3), np.float32)
    log_det = np.empty((B,), np.float32)
    for c in range(NCORES):
        o = res.results[c]["outT"]                       # [3, NB]
        out[c * BPC:(c + 1) * BPC] = o.reshape(3, BPC, N).transpose(1, 2, 0)
        log_det[c * BPC:(c + 1) * BPC] = res.results[c]["logdet"][0]
    return out, log_det
